# revision 8
# baseline (speedup 1.0000x reference)
"""Trainium2 Bass kernel for the BoundaryCorrectionModule problem.

Full inputs in, full output out. Internally: pure data-parallel over the
batch dim across 8 NeuronCores (2048 rows each). Activations live in
SBUF in feature-major ("transposed") layout [128p, 8kc, batch], bf16.
Weights are host-transposed and packed so every device DMA is a single
contiguous read. All concat-matmuls are decomposed into 1024x1024 units
accumulated in PSUM (f32); ACT evacuates PSUM with fused bias +
sigmoid/tanh; DVE applies the gating / GRU arithmetic.

Step-0 algebraic fold: S0 = 0.5*(h_prev+h_next) = 0.5*(M+D), so step 0
uses W_SM' = W_SM + 0.5*W_SS and W_SD' = W_SD + 0.5*W_SS and never
materializes S0 (saves one full D x D unit-GEMM per step-0).
"""

import numpy as np
import ml_dtypes

import concourse.bass as bass
import concourse.mybir as mybir
import concourse.tile as tile
from concourse import bacc
from concourse.bass_utils import run_bass_kernel_spmd

BF16 = ml_dtypes.bfloat16
F32 = np.float32

B = 16384          # full batch
D = 1024           # feature dim
NCORES = 8
BC = B // NCORES   # per-core batch (2048)
NHALF = 2          # batch sub-passes per core (weights re-streamed per pass)
H = BC // NHALF    # rows per pass (1024)
FREE = 512         # matmul moving free dim (ISA cap; one PSUM bank of f32)
NB = H // FREE     # moving tiles per pass
KC = D // 128      # contraction chunks (8)
MC = D // 128      # output-feature chunks (8)
NSTEP = 2

# unit ids (each is a [D, D] block of some weight, pre-transposed to
# lhsT layout on the host)
U_SS, U_SM, U_SD = 0, 1, 2
U_gS_M, U_gS_D = 3, 4
U_Mz_S, U_Mz_M, U_Mz_D = 5, 6, 7
U_Mr_S, U_Mr_M, U_Mr_D = 8, 9, 10
U_Mh_S, U_Mh_rM, U_Mh_D = 11, 12, 13
U_DS, U_DM, U_DD = 14, 15, 16
U_gD_S, U_gD_M = 17, 18
U_SM0, U_SD0 = 19, 20      # step-0 folded: W_SM + 0.5 W_SS, W_SD + 0.5 W_SS
NUNITS = 21

# bias ids
B_gS, B_Mz, B_Mr, B_Mh, B_gD = 0, 1, 2, 3, 4
NBIAS = 5

SIG = mybir.ActivationFunctionType.Sigmoid
TANH = mybir.ActivationFunctionType.Tanh

_BUILD_CACHE = {}


def _pack_unit(wt_block):
    """[D(k), D(m)] f32 (already W.T) -> [MC, 128, KC, 128] bf16 contiguous,
    laid out exactly as the SBUF weight tile [p, kc, m] per mc."""
    w = wt_block.reshape(KC, 128, MC, 128)          # (kc, p, mc, m)
    w = np.transpose(w, (2, 1, 0, 3))               # (mc, p, kc, m)
    return np.ascontiguousarray(w.astype(BF16))


def _pack_acts(hT_core):
    """[D, BC] f32 (feature-major slice for one core) ->
    [NHALF, 128, KC, H] bf16 contiguous (per-pass SBUF tile layout)."""
    x = hT_core.reshape(KC, 128, BC)                # (kc, p, b)
    x = np.transpose(x, (1, 0, 2))                  # (p, kc, b)
    x = x.reshape(128, KC, NHALF, H)
    x = np.transpose(x, (2, 0, 1, 3))               # (h, p, kc, b)
    return np.ascontiguousarray(x.astype(BF16))


def _build():
    """Build + compile the Bass module once per process."""
    key = (BC, FREE)
    if key in _BUILD_CACHE:
        return _BUILD_CACHE[key]

    nc = bacc.Bacc("TRN2", target_bir_lowering=False, debug=False)
    bf = mybir.dt.bfloat16
    f32 = mybir.dt.float32

    hp_d = nc.dram_tensor("hp", [NHALF, 128, KC, H], bf, kind="ExternalInput")
    hn_d = nc.dram_tensor("hn", [NHALF, 128, KC, H], bf, kind="ExternalInput")
    wu_d = nc.dram_tensor("wu", [NUNITS, MC, 128, KC, 128], bf, kind="ExternalInput")
    bias_d = nc.dram_tensor("bias", [128, NBIAS, MC], f32, kind="ExternalInput")
    rs_d = nc.dram_tensor("rs", [128, 1], f32, kind="ExternalInput")
    out_d = nc.dram_tensor("out", [NHALF, MC, 128, H], f32, kind="ExternalOutput")

    with tile.TileContext(nc) as tc:
        with (
            tc.tile_pool(name="const", bufs=1) as const_p,
            tc.tile_pool(name="st", bufs=2) as st_p,
            tc.tile_pool(name="aux", bufs=1) as aux_p,
            tc.tile_pool(name="wp", bufs=12) as w_p,
            tc.tile_pool(name="tp", bufs=6) as t_p,
            tc.tile_pool(name="dp", bufs=4) as d_p,
            tc.tile_pool(name="op", bufs=4) as o_p,
            tc.tile_pool(name="ps", bufs=8, space="PSUM") as ps_p,
        ):
            bias_t = const_p.tile([128, NBIAS, MC], f32)
            nc.sync.dma_start(bias_t[:], bias_d.ap()[:, :, :])
            rs_t = const_p.tile([128, 1], f32)
            nc.sync.dma_start(rs_t[:], rs_d.ap()[:, :])

            def load_w(u, mc):
                w = w_p.tile([128, KC, 128], bf, tag="w", name=f"w{u}_{mc}")
                nc.sync.dma_start(w[:], wu_d.ap()[u, mc])
                return w

            def phase(units, bias_idx, evac, preloaded=None):
                """One matmul phase over the full pass batch.

                units: list of (unit_id, src_tile); accumulated in PSUM.
                evac(psum, mc, n, bias_ap) consumes each PSUM sub-tile.
                """
                for mc in range(MC):
                    wts = []
                    for (u, src) in units:
                        if preloaded and (u, mc) in preloaded:
                            w = preloaded[(u, mc)]
                        else:
                            w = load_w(u, mc)
                        wts.append((w, src))
                    psums = []
                    for n in range(NB):
                        p = ps_p.tile([128, FREE], f32, tag="p", name=f"p{mc}_{n}")
                        psums.append(p)
                    total = len(units) * KC
                    i = 0
                    for (w, src) in wts:
                        for kc in range(KC):
                            for n in range(NB):
                                nc.tensor.matmul(
                                    psums[n][:, :],
                                    w[:, kc, :],
                                    src[:, kc, bass.ts(n, FREE)],
                                    start=(i == 0),
                                    stop=(i == total - 1),
                                )
                            i += 1
                    b_ap = bias_t[:, bias_idx, mc:mc + 1]
                    for n in range(NB):
                        evac(psums[n], mc, n, b_ap)

            def evac_plain(dst, func):
                def f(psum, mc, n, b_ap):
                    nc.scalar.activation(
                        dst[:, mc, bass.ts(n, FREE)], psum[:, :], func, bias=b_ap)
                return f

            def evac_gated(dst, func, gate):
                def f(psum, mc, n, b_ap):
                    t = t_p.tile([128, FREE], bf, tag="t", name=f"t{mc}_{n}")
                    nc.scalar.activation(t[:], psum[:, :], func, bias=b_ap)
                    nc.vector.tensor_mul(
                        dst[:, mc, bass.ts(n, FREE)], t[:],
                        gate[:, mc, bass.ts(n, FREE)])
                return f

            def evac_gru(dst, M_old, z):
                def f(psum, mc, n, b_ap):
                    sl = bass.ts(n, FREE)
                    t = t_p.tile([128, FREE], bf, tag="t", name=f"t{mc}_{n}")
                    nc.scalar.activation(t[:], psum[:, :], TANH, bias=b_ap)
                    d = d_p.tile([128, FREE], bf, tag="d", name=f"d{mc}_{n}")
                    nc.vector.tensor_sub(d[:], t[:], M_old[:, mc, sl])
                    nc.vector.tensor_mul(d[:], d[:], z[:, mc, sl])
                    nc.vector.tensor_add(dst[:, mc, sl], M_old[:, mc, sl], d[:])
                return f

            for h in range(NHALF):
                M = st_p.tile([128, KC, H], bf, tag="M", name=f"M_{h}")
                Dv = st_p.tile([128, KC, H], bf, tag="D", name=f"D_{h}")
                if h == 0:
                    # Get the first phase's weights onto the (FIFO) DMA ring
                    # before the 4MB of activations so PE can start early.
                    pre = {(U_gS_M, 0): load_w(U_gS_M, 0),
                           (U_gS_D, 0): load_w(U_gS_D, 0)}
                    # mc=0 weights first on the FIFO DMA ring, then M before
                    # Dv, each as one fully-contiguous 2MB transfer.
                    nc.sync.dma_start(M[:], hp_d.ap()[h])
                    nc.sync.dma_start(Dv[:], hn_d.ap()[h])
                else:
                    pre = None
                    nc.sync.dma_start(M[:], hp_d.ap()[h])
                    nc.sync.dma_start(Dv[:], hn_d.ap()[h])
                S = None

                for step in range(NSTEP):
                    last = step == NSTEP - 1

                    GS = aux_p.tile([128, KC, H], bf, tag="GS", name=f"GS_{h}_{step}")
                    phase([(U_gS_M, M), (U_gS_D, Dv)], B_gS,
                          evac_plain(GS, SIG), preloaded=pre)
                    pre = None

                    S_new = st_p.tile([128, KC, H], bf, tag="S", name=f"Sn_{h}_{step}")
                    if step == 0:
                        s_units = [(U_SM0, M), (U_SD0, Dv)]
                    else:
                        s_units = [(U_SS, S), (U_SM, M), (U_SD, Dv)]
                    phase(s_units, B_gS, evac_gated(S_new, TANH, GS))

                    z = aux_p.tile([128, KC, H], bf, tag="z", name=f"z_{h}_{step}")
                    phase([(U_Mz_S, S_new), (U_Mz_M, M), (U_Mz_D, Dv)], B_Mz,
                          evac_plain(z, SIG))

                    rM = aux_p.tile([128, KC, H], bf, tag="rM", name=f"rM_{h}_{step}")
                    phase([(U_Mr_S, S_new), (U_Mr_M, M), (U_Mr_D, Dv)], B_Mr,
                          evac_gated(rM, SIG, M))

                    M_new = st_p.tile([128, KC, H], bf, tag="M", name=f"Mn_{h}_{step}")
                    phase([(U_Mh_S, S_new), (U_Mh_rM, rM), (U_Mh_D, Dv)], B_Mh,
                          evac_gru(M_new, M, z))

                    GD = aux_p.tile([128, KC, H], bf, tag="GD", name=f"GD_{h}_{step}")
                    phase([(U_gD_S, S_new), (U_gD_M, M_new)], B_gD,
                          evac_plain(GD, SIG))

                    d_units = [(U_DS, S_new), (U_DM, M_new), (U_DD, Dv)]
                    if not last:
                        D_new = st_p.tile([128, KC, H], bf, tag="D",
                                          name=f"Dn_{h}_{step}")
                        phase(d_units, B_gD, evac_gated(D_new, TANH, GD))
                        S, M, Dv = S_new, M_new, D_new
                    else:
                        # Fused tail: D_new = tanh(.)*GD exists only per-chunk;
                        # out = M_new + rs*(S_new + D_new) streams straight out.
                        def evac_final(psum, mc, n, b_ap,
                                       _S=S_new, _M=M_new, _GD=GD, _h=h):
                            sl = bass.ts(n, FREE)
                            t = t_p.tile([128, FREE], bf, tag="t",
                                         name=f"t{mc}_{n}")
                            nc.scalar.activation(t[:], psum[:, :], TANH, bias=b_ap)
                            d = d_p.tile([128, FREE], bf, tag="d",
                                         name=f"d{mc}_{n}")
                            nc.vector.tensor_mul(d[:], t[:], _GD[:, mc, sl])
                            o = o_p.tile([128, FREE], f32, tag="o",
                                         name=f"o_{_h}_{mc}_{n}")
                            nc.vector.tensor_add(o[:], _S[:, mc, sl], d[:])
                            nc.vector.tensor_scalar_mul(o[:], o[:], rs_t[:, 0:1])
                            nc.vector.tensor_add(o[:], o[:], _M[:, mc, sl])
                            nc.sync.dma_start(
                                out_d.ap()[_h, mc, :, bass.ts(n, FREE)], o[:])
                        phase(d_units, B_gD, evac_final)

    nc.compile()
    _BUILD_CACHE[key] = nc
    return nc


def _pack_inputs(h_prev, h_next, W_SS, W_SM, W_SD, W_Mz, b_Mz, W_Mr, b_Mr,
                 W_Mh, b_Mh, W_DS, W_DM, W_DD, W_gS, b_gS, W_gD, b_gD,
                 residual_scale):
    """Host-side packing: transposes, bf16 casts, per-core sharding."""
    units = [None] * NUNITS
    f = np.float32

    def T(w):
        return np.ascontiguousarray(np.asarray(w, f).T)

    t_ss, t_sm, t_sd = T(W_SS), T(W_SM), T(W_SD)
    units[U_SS] = _pack_unit(t_ss)
    units[U_SM] = _pack_unit(t_sm)
    units[U_SD] = _pack_unit(t_sd)
    units[U_SM0] = _pack_unit(t_sm + f(0.5) * t_ss)
    units[U_SD0] = _pack_unit(t_sd + f(0.5) * t_ss)
    gs = T(W_gS)                       # [2D, D]
    units[U_gS_M] = _pack_unit(gs[:D])
    units[U_gS_D] = _pack_unit(gs[D:])
    for base, Wx in ((U_Mz_S, W_Mz), (U_Mr_S, W_Mr), (U_Mh_S, W_Mh)):
        wx = T(Wx)                     # [3D, D]
        units[base] = _pack_unit(wx[:D])
        units[base + 1] = _pack_unit(wx[D:2 * D])
        units[base + 2] = _pack_unit(wx[2 * D:])
    units[U_DS] = _pack_unit(T(W_DS))
    units[U_DM] = _pack_unit(T(W_DM))
    units[U_DD] = _pack_unit(T(W_DD))
    gd = T(W_gD)
    units[U_gD_S] = _pack_unit(gd[:D])
    units[U_gD_M] = _pack_unit(gd[D:])
    wu = np.stack(units)               # [NUNITS, MC, 128, KC, 128] bf16

    bias = np.stack([np.asarray(b, f) for b in (b_gS, b_Mz, b_Mr, b_Mh, b_gD)])
    bias = bias.reshape(NBIAS, MC, 128)
    bias = np.ascontiguousarray(np.transpose(bias, (2, 0, 1)))  # [128, NBIAS, MC]

    rs = np.full((128, 1), np.asarray(residual_scale, f), dtype=f)

    hpT = np.asarray(h_prev, f).T      # [D, B] view
    hnT = np.asarray(h_next, f).T

    in_maps = []
    for c in range(NCORES):
        sl = slice(c * BC, (c + 1) * BC)
        in_maps.append({
            "hp": _pack_acts(np.ascontiguousarray(hpT[:, sl])),
            "hn": _pack_acts(np.ascontiguousarray(hnT[:, sl])),
            "wu": wu,
            "bias": bias,
            "rs": rs,
        })
    return in_maps


def _unpack_output(results):
    """Per-core [NHALF, MC, 128, H] f32 -> full [B, D] f32."""
    blocks = []
    for c in range(NCORES):
        a = results[c]["out"]                       # [NHALF, MC, 128, H]
        a = np.transpose(a, (1, 2, 0, 3)).reshape(D, BC)  # feature-major
        blocks.append(a)
    outT = np.concatenate(blocks, axis=1)           # [D, B]
    return np.ascontiguousarray(outT.T)


def run(trace=False, tmpdir=None, trace_kwargs=None, **inputs):
    """Extended entry point: returns (output, BassKernelResults)."""
    nc = _build()
    in_maps = _pack_inputs(**inputs)
    res = run_bass_kernel_spmd(
        nc, in_maps, core_ids=list(range(NCORES)),
        trace=trace, tmpdir=tmpdir, **(trace_kwargs or {}))
    return _unpack_output(res.results), res


def kernel(**inputs):
    out, _ = run(**inputs)
    return out
